# revision 55
# baseline (speedup 1.0000x reference)
"""Trainium2 Bass kernel for nn_Decoder: attention+LSTM decoder (v2).

Math (reference):
  k = h_enc @ Wk.T + bk ; v = h_enc @ Wv.T + bv        [B, 8, 32]
  3 decoder steps: q = h @ Wq.T + bq
     score_t = q.k_t/sqrt(32) ; att = softmax_t
     ctx = sum_t att_t v_t ; (h, c) = LSTMCell(ctx, h, c)
  logits_s = h_s @ Wout.T + b_out ; out = log_softmax(logits)   [B, 3, 10]

Kernel algebra (host-side folds):
  score_t = h.(A x_t) + w.x_t  with A = Wq.T Wk/sqrt(H), w = Wk.T bq/sqrt(H)
  es1 = exp(w.x) is the full step-1 softmax numerator (h0 = 0); for s>1
  es_s = exp(h.k~) * es1, so the step-1 products q1 = es1*v~ and es1 are
  stashed and reused (no per-step recompute of w.x).
  v-bias bv folded into gate bias: bg = b_ih + b_hh + W_ih @ bv
  sigmoid via tanh: sig(x) = (1+tanh(x/2))/2; factor-2 carries:
     Chat = 2c, Hhat = 2h; consumers of Hhat pre-scaled by 0.5.
  ssum contraction computes 32*sum_t(e_t); the 32 folded into W_ih (x32).

Layout: feature-major t-packed [128, 2n] tiles: partition = 32*(t%4)+h,
free = (half, batch): cols 0:n = t0-3, n:2n = t4-7.  n = CHUNK.
All heavy matmuls are single instructions with block-diagonal / stacked
weights (full 128-row streams) instead of 32x32 tile_position volleys.
LSTM state per step s lives in S_s [64, n]: band0 = Hhat_{s-1}, band1 = cx_s,
so the gates matmul contracts K=64 contiguously.  Chat' stays in PSUM
between steps (mm2 reads it as the PSUM operand of an STT).
Phase B split: B1 (exp table) emits eo/lgs per chunk into a group stash;
B2 (ln table) runs once per GROUP -> only 2 act-table swaps per group.
"""

import numpy as np

import concourse.bass as bass
import concourse.bacc as bacc
import concourse.tile as tile
from concourse import mybir
from concourse.bass_utils import run_bass_kernel_spmd

H = 32
HT = 8
FT = 3
OD = 10
N_CORES = 8

BF = mybir.dt.bfloat16
F32 = mybir.dt.float32
AF = mybir.ActivationFunctionType
ALU = mybir.AluOpType

CHUNK = 512          # batch elements per chunk
GROUP = 8            # chunks per phase-B2 stash group

# wpack (bf16, [128, WCOLS]) column layout
ID128_OFF = 0        # 128: identity (input transposes)
BDA_OFF = 128        # 128: blockdiag(A.T) x4
BDV_OFF = 256        # 128: blockdiag(Wv.T) x4
BDK_OFF = 384        # 128: blockdiag(tile(w)) x4  (score bias w.x)
BDO_OFF = 512        # 128: blockdiag(ones32) x4   (score contraction)
REP_OFF = 640        # 128 (rows 0:32): 0.5 * h-replicator to 4 bands
G1_OFF = 768         # 128 (rows 32:64): (32*W_ih).T cols {i,f,o,g} (s=1, cx only)
G3_OFF = 896         # 128 (rows 0:64): {(0.5*W_hh).T; (32*W_ih).T} cols {i,f,o,g}
I32S_OFF = 1024      # 32: stacked eye(32) [128,32]  (context contraction)
ONES_OFF = 1056      # 32: ones [128,32]             (ssum contraction)
WOUT_OFF = 1088      # 32 (rows 0:32): (0.5*W_out).T padded
BDO96_OFF = 1120     # 96 (rows 0:96): blockdiag(ones32) x3 (phase-B ssum)
WCOLS = 1216

# fpack (f32, [128, FCOLS])
SV_OFF = 0           # rows 0:96 = 0.5 (tanh scale for i,f,o bands)
BT_OFF = 1           # rows 0:96 = 0.5*bg_{i,f,o}; rows 96:128 = bg_g
BOUT_OFF = 2         # rows 0:96: b_out padded (-30) per 3 s-blocks
GSV_OFF = 3          # gtc2 pair-tanh scale: {1, 1, .5, .5} per band
GBT_OFF = 4          # gtc2 pair-tanh bias: {bg_g, bg_g, 0, 0}
IDT_OFF = 5          # 96: f32 identity[96] (output transposes)
IDF_OFF = 5 + 96     # 128: f32 identity[128] (input transposes, casting)
FCOLS = 5 + 96 + 128


def _pack_weights(Wq, bq, Wk, bk, Wv, bv, W_ih, b_ih, W_hh, b_hh, W_out, b_out):
    Wq, bq, Wk, bk, Wv, bv, W_ih, b_ih, W_hh, b_hh, W_out, b_out = [
        np.asarray(a, np.float32) for a in
        (Wq, bq, Wk, bk, Wv, bv, W_ih, b_ih, W_hh, b_hh, W_out, b_out)]
    s = 1.0 / np.sqrt(np.float32(H))
    A = (Wq.T @ Wk) * s                    # [32,32] score bilinear form
    w = (Wk.T @ bq) * s                    # [32]
    bg = b_ih + b_hh + W_ih @ bv           # [128] gate bias (i,f,g,o order)

    eye = np.eye(32, dtype=np.float32)
    wp = np.zeros((128, WCOLS), np.float32)
    wp[:, ID128_OFF:ID128_OFF + 128] = np.eye(128)
    for r in range(4):
        P = slice(32 * r, 32 * r + 32)
        C = slice(32 * r, 32 * r + 32)
        wp[P, BDA_OFF + 32 * r:BDA_OFF + 32 * r + 32] = A.T
        wp[P, BDV_OFF + 32 * r:BDV_OFF + 32 * r + 32] = Wv.T
        wp[P, BDK_OFF + 32 * r:BDK_OFF + 32 * r + 32] = np.tile(w[:, None], (1, 32))
        wp[P, BDO_OFF + 32 * r:BDO_OFF + 32 * r + 32] = 1.0
        wp[P, I32S_OFF:I32S_OFF + 32] = eye
        wp[P, ONES_OFF:ONES_OFF + 32] = 1.0
        wp[0:32, REP_OFF + 32 * r:REP_OFF + 32 * r + 32] = 0.5 * eye
    # gate col order {i,f,o,g}: source rows of W_ih/W_hh: i 0:32, f 32:64,
    # g 64:96, o 96:128
    gate_slices = (slice(0, 32), slice(32, 64), slice(96, 128), slice(64, 96))
    for ci, gsl in enumerate(gate_slices):
        # G1 rows 32:64: s=1 gates contract cx1 at S[0][32:64]
        wp[32:64, G1_OFF + 32 * ci:G1_OFF + 32 * ci + 32] = (32.0 * W_ih[gsl]).T
        wp[0:32, G3_OFF + 32 * ci:G3_OFF + 32 * ci + 32] = (0.5 * W_hh[gsl]).T
        wp[32:64, G3_OFF + 32 * ci:G3_OFF + 32 * ci + 32] = (32.0 * W_ih[gsl]).T
    wp[0:32, WOUT_OFF:WOUT_OFF + OD] = (0.5 * W_out).T
    for r in range(3):
        wp[32 * r:32 * r + 32, BDO96_OFF + 32 * r:BDO96_OFF + 32 * r + 32] = 1.0

    fp = np.zeros((128, FCOLS), np.float32)
    fp[0:96, SV_OFF] = 0.5
    fp[0:96, BT_OFF] = 0.5 * np.concatenate([bg[0:32], bg[32:64], bg[96:128]])
    fp[96:128, BT_OFF] = bg[64:96]
    bout = np.full(96, -30.0, np.float32)
    for s3 in range(FT):
        bout[32 * s3:32 * s3 + OD] = b_out
    fp[0:96, BOUT_OFF] = bout
    fp[0:64, GSV_OFF] = 1.0
    fp[64:128, GSV_OFF] = 0.5
    fp[0:32, GBT_OFF] = bg[64:96]
    fp[32:64, GBT_OFF] = bg[64:96]
    fp[0:96, IDT_OFF:IDT_OFF + 96] = np.eye(96)
    fp[:, IDF_OFF:IDF_OFF + 128] = np.eye(128)
    return wp, fp


def build_program(Bshard: int, debug: bool = False) -> bass.Bass:
    assert Bshard % (CHUNK * GROUP) == 0
    nchunks = Bshard // CHUNK
    nc = bacc.Bacc(trn_type="TRN2")
    x_d = nc.declare_dram_parameter("h_enc", [Bshard, HT, H], F32, isOutput=False)
    wp_d = nc.declare_dram_parameter("wpack", [128, WCOLS], BF, isOutput=False)
    fp_d = nc.declare_dram_parameter("fpack", [128, FCOLS], F32, isOutput=False)
    out_d = nc.declare_dram_parameter("out", [Bshard, FT, OD], F32, isOutput=True)
    dbg = None
    if debug:
        dbg = {nm: nc.declare_dram_parameter(nm, shp, F32, isOutput=True)
               for nm, shp in [("d_xs", [128, 2 * CHUNK]),
                               ("d_ks", [128, 2 * CHUNK]),
                               ("d_vs", [128, 2 * CHUNK]),
                               ("d_es1", [128, 2 * CHUNK]),
                               ("d_cusm1", [128, CHUNK]),
                               ("d_tt1", [96, CHUNK]),
                               ("d_qt1", [128, 2 * CHUNK]),
                               ("d_g2o1", [64, CHUNK]),
                               ("d_cx1", [32, CHUNK]),
                               ("d_S1", [96, CHUNK]),
                               ("d_S2", [96, CHUNK]),
                               ("d_S3", [96, CHUNK])]}
    with tile.TileContext(nc) as tc:
        _body(nc, tc, x_d, wp_d, fp_d, out_d, nchunks, CHUNK, dbg)
    nc.compile()
    return nc


def _body(nc, tc, x_d, wp_d, fp_d, out_d, nchunks, n, dbg=None):
    from contextlib import ExitStack
    n2 = 2 * n
    ctx = ExitStack()
    with ctx:
        singles = ctx.enter_context(tc.tile_pool(name="singles", bufs=1))
        sb_xb = ctx.enter_context(tc.tile_pool(name="sb_xb", bufs=2, ))
        sb_xs = ctx.enter_context(tc.tile_pool(name="sb_xs", bufs=8, ))
        sb_kq = ctx.enter_context(tc.tile_pool(name="sb_kq", bufs=9))
        sb_step = ctx.enter_context(tc.tile_pool(name="sb_step", bufs=3, ))
        sb_b2 = ctx.enter_context(tc.tile_pool(name="sb_b2", bufs=2))
        sb_sm = ctx.enter_context(tc.tile_pool(name="sb_sm", bufs=3))
        sb_cht = ctx.enter_context(tc.tile_pool(name="sb_cht", bufs=6))
        sb_st = ctx.enter_context(tc.tile_pool(name="sb_st", bufs=5, ))
        sb_out = ctx.enter_context(tc.tile_pool(name="sb_out", bufs=2, ))
        # PSUM (8 banks):
        #  ps_big: xp/kp/vp/scp/hrp [128,2n] f32 2 banks, bufs 2 -> 4
        #  ps_step: cusm/gp/g2/g2o/lg/so/ot <=1 bank, bufs 4     -> 4
        ps_big = ctx.enter_context(tc.tile_pool(name="ps_big", bufs=2, space="PSUM"))
        ps_step = ctx.enter_context(tc.tile_pool(name="ps_step", bufs=2, space="PSUM"))
        ps_gp = ctx.enter_context(tc.tile_pool(name="ps_gp", bufs=1, space="PSUM"))

        wp = singles.tile([128, WCOLS], BF)
        nc.sync.dma_start(out=wp, in_=wp_d[:, :])
        fp = singles.tile([128, FCOLS], F32)
        nc.sync.dma_start(out=fp, in_=fp_d[:, :])

        def mm(out_ps, lhsT, rhs, start=True, stop=True, pos=(0, 0)):
            nc.tensor.matmul(out_ps, lhsT, rhs, start=start, stop=stop,
                             tile_position=pos, skip_group_check=True)

        ngroups = (nchunks + GROUP - 1) // GROUP
        npairs = GROUP // 2
        nsub = n // 128
        for gi in range(ngroups):
            g0 = gi * GROUP
            stash_eo = sb_out.tile([96, GROUP * n], BF, tag="stash_eo")
            stash_lg = sb_out.tile([96, GROUP * n], BF, tag="stash_lg")

            # ---- prep: first two pairs now, the rest interleaved ----
            xs_g, ks_g, vs_g, S_g = [], [], [], []

            def prep_chunk(cj):
                ci = g0 + cj
                xb = sb_xb.tile([128, nsub, 256], F32, tag="xb", name="xb")
                xv = x_d[ci * n:(ci + 1) * n].rearrange(
                    "(i p) t h -> p i (t h)", p=128)
                nc.sync.dma_start(out=xb, in_=xv)
                xp = ps_big.tile([128, n2], F32, tag="big", name="xp")
                for half in range(2):
                    for i in range(nsub):
                        nc.tensor.transpose(
                            xp[:, n * half + 128 * i:n * half + 128 * i + 128],
                            xb[:, i, 128 * half:128 * half + 128],
                            fp[:, IDF_OFF:IDF_OFF + 128])
                xs = sb_xs.tile([128, n2], BF, tag="xs", name="xs")
                nc.scalar.copy(out=xs, in_=xp)
                xs_g.append(xs)
                if dbg is not None and gi == 0 and cj == 0:
                    nc.gpsimd.dma_start(out=dbg["d_xs"][:, :], in_=xs)
                kp = ps_big.tile([128, n2], F32, tag="big", name="kp")
                for half in range(2):
                    mm(kp[:, n * half:n * half + n],
                       wp[:, BDA_OFF:BDA_OFF + 128],
                       xs[:, n * half:n * half + n])
                ks = sb_kq.tile([128, n2], BF, tag="ks", name="ks")
                nc.vector.tensor_copy(ks, kp)
                ks_g.append(ks)
                vp = ps_big.tile([128, n2], F32, tag="big", name="vp")
                for half in range(2):
                    mm(vp[:, n * half:n * half + n],
                       wp[:, BDV_OFF:BDV_OFF + 128],
                       xs[:, n * half:n * half + n])
                vs = sb_kq.tile([128, n2], BF, tag="vs", name="vs")
                nc.scalar.copy(out=vs, in_=vp)
                vs_g.append(vs)
                if dbg is not None and gi == 0 and cj == 0:
                    nc.gpsimd.dma_start(out=dbg["d_ks"][:, :], in_=ks)
                    nc.gpsimd.dma_start(out=dbg["d_vs"][:, :], in_=vs)
                # S[s] [96, n]: {Hhat_s @0:32, Chat_s @32:64, cx_{s+1} @64:96}
                S_g.append([sb_st.tile([64, n], BF, tag=f"S{cj % 2}_{s}",
                                       name=f"S{s}") for s in range(4)])

            for cj in range(4):
                prep_chunk(cj)

            # ---- steps, software-pipelined across the group's pairs ----
            cht_prev_d = {}
            for s in range(1, FT + 1):
                for pj in range(npairs):
                    if s == 1 and 4 + 2 * pj < GROUP:
                        prep_chunk(4 + 2 * pj)
                        prep_chunk(5 + 2 * pj)
                    cjs = (2 * pj, 2 * pj + 1)
                    ep_p, qt_p = [], []
                    for c in cjs:
                        xs, ks, S = xs_g[c], ks_g[c], S_g[c]
                        scp = ps_big.tile([128, n2], F32, tag="big",
                                          name="scp")
                        if s == 1:
                            for half in range(2):
                                mm(scp[:, n * half:n * half + n],
                                   wp[:, BDK_OFF:BDK_OFF + 128],
                                   xs[:, n * half:n * half + n])
                        else:
                            hrp = ps_big.tile([128, n2], F32, tag="big",
                                              name="hrp")
                            for half in range(2):
                                mm(hrp[:, n * half:n * half + n],
                                   wp[0:32, REP_OFF:REP_OFF + 128],
                                   S[s - 1][0:32, :])
                            ptt = sb_step.tile([128, n2], BF, tag="ptt",
                                               name="ptt", bufs=2)
                            nc.vector.tensor_mul(ptt, ks, hrp)
                            for half in range(2):
                                mm(scp[:, n * half:n * half + n],
                                   wp[:, BDK_OFF:BDK_OFF + 128],
                                   xs[:, n * half:n * half + n],
                                   start=True, stop=False)
                                mm(scp[:, n * half:n * half + n],
                                   wp[:, BDO_OFF:BDO_OFF + 128],
                                   ptt[:, n * half:n * half + n],
                                   start=False, stop=True)
                        ep = sb_step.tile([128, n2], BF, tag="ep", name="ep")
                        nc.scalar.activation(out=ep, in_=scp, func=AF.Exp)
                        ep_p.append(ep)
                        if dbg is not None and gi == 0 and c == 0 and s == 1:
                            nc.gpsimd.dma_start(out=dbg["d_es1"][:, :], in_=ep)
                        qt = sb_step.tile([128, n2], BF, tag="qt", name="qt")
                        nc.vector.tensor_mul(qt, ep, vs_g[c])
                        qt_p.append(qt)
                    # shared pair psum: cu_a, cu_b, ssum_a, ssum_b
                    # ssum at [0:64] (recip_approx needs base 0), cu at [64:128]
                    cusm = ps_step.tile([128, n], F32, tag="stp", name="cusm")
                    for idx in range(2):
                        for half in range(2):
                            mm(cusm[32 * idx:32 * idx + 32, :],
                               wp[:, ONES_OFF:ONES_OFF + 32],
                               ep_p[idx][:, n * half:n * half + n],
                               start=(half == 0), stop=(half == 1),
                               pos=(0, 32 * idx))
                        for half in range(2):
                            mm(cusm[64 + 32 * idx:96 + 32 * idx, :],
                               wp[:, I32S_OFF:I32S_OFF + 32],
                               qt_p[idx][:, n * half:n * half + n],
                               start=(half == 0), stop=(half == 1),
                               pos=(0, 64 + 32 * idx))
                    if dbg is not None and gi == 0 and pj == 0 and s == 1:
                        dcu = sb_sm.tile([128, n], F32, tag="dcu", name="dcu")
                        nc.vector.tensor_copy(dcu, cusm)
                        nc.sync.dma_start(out=dbg["d_cusm1"][:, :], in_=dcu)
                        nc.gpsimd.dma_start(out=dbg["d_qt1"][:, :], in_=qt_p[0])
                    rs = sb_b2.tile([64, n], F32, tag="rs", name="rs")
                    nc.vector.reciprocal_approx_fast(out=rs,
                                                     in_=cusm[0:64, :])

                    # pair-packed gates: gp2 [128, 2n], chunk idx at free half
                    gp2 = ps_gp.tile([128, n2], F32, tag="gp2", name="gp2")
                    for idx, c in enumerate(cjs):
                        S = S_g[c]
                        # cx_s -> S[s-1][32:64]
                        nc.vector.tensor_tensor(
                            out=S[s - 1][32:64, :],
                            in0=cusm[64 + 32 * idx:96 + 32 * idx, :],
                            in1=rs[32 * idx:32 * idx + 32, :],
                            op=ALU.mult)
                        if s == 1:
                            mm(gp2[:, idx * n:idx * n + n],
                               wp[32:64, G1_OFF:G1_OFF + 128],
                               S[0][32:64, :], pos=(32, 0))
                        else:
                            mm(gp2[:, idx * n:idx * n + n],
                               wp[0:64, G3_OFF:G3_OFF + 128],
                               S[s - 1][0:64, :])
                    tt2 = sb_step.tile([96, n2], BF, tag="tt2", name="tt2")
                    nc.scalar.activation(
                        out=tt2, in_=gp2[0:96, :], func=AF.Tanh,
                        scale=fp[0:96, SV_OFF:SV_OFF + 1],
                        bias=fp[0:96, BT_OFF:BT_OFF + 1])
                    g2o = sb_sm.tile([32, n2], BF, tag="g2o", name="g2o")
                    nc.scalar.activation(out=g2o, in_=gp2[96:128, :],
                                         func=AF.Tanh,
                                         bias=fp[96:128, BT_OFF:BT_OFF + 1])
                    # cht2 [64, 2n]: Chat at [32:64] (aligned with tt2 f-band)
                    cht2 = sb_cht.tile([64, n2], BF, tag="cht", name="cht2")
                    if s == 1:
                        nc.vector.scalar_tensor_tensor(
                            out=cht2[32:64, :], in0=tt2[0:32, :], scalar=1.0,
                            in1=g2o, op0=ALU.add, op1=ALU.mult)
                    else:
                        m1t = sb_sm.tile([32, n2], BF, tag="m1t", name="m1t")
                        nc.vector.scalar_tensor_tensor(
                            out=m1t, in0=tt2[0:32, :], scalar=1.0,
                            in1=g2o, op0=ALU.add, op1=ALU.mult)
                        m2t = sb_sm.tile([32, n2], BF, tag="m2t", name="m2t")
                        nc.vector.scalar_tensor_tensor(
                            out=m2t, in0=tt2[32:64, :], scalar=1.0,
                            in1=cht_prev_d[pj][32:64, :],
                            op0=ALU.add, op1=ALU.mult)
                        nc.vector.scalar_tensor_tensor(
                            out=cht2[32:64, :], in0=m2t, scalar=0.5,
                            in1=m1t, op0=ALU.mult, op1=ALU.add)
                    if dbg is not None and gi == 0 and pj == 0 and s == 1:
                        nc.gpsimd.dma_start(out=dbg["d_tt1"][:, :],
                                            in_=tt2[:, 0:n])
                        nc.gpsimd.dma_start(out=dbg["d_g2o1"][0:32, :],
                                            in_=g2o[:, 0:n])
                        nc.gpsimd.dma_start(out=dbg["d_cx1"][:, :],
                                            in_=cht2[32:64, 0:n])
                    tct2 = sb_sm.tile([96, n2], BF, tag="tct2", name="tct2")
                    nc.scalar.activation(out=tct2[64:96, :],
                                         in_=cht2[32:64, :],
                                         func=AF.Tanh, scale=0.5)
                    for idx, c in enumerate(cjs):
                        nc.vector.scalar_tensor_tensor(
                            out=S_g[c][s][0:32, :],
                            in0=tt2[64:96, idx * n:idx * n + n], scalar=1.0,
                            in1=tct2[64:96, idx * n:idx * n + n],
                            op0=ALU.add, op1=ALU.mult)
                    cht_prev_d[pj] = cht2

            # ---- B1: logits -> eo/lgs group stash ----
            for cj in range(GROUP):
                lg = ps_step.tile([96, n], F32, tag="stp", name="lg")
                for s3 in range(FT):
                    mm(lg[32 * s3:32 * s3 + 32, :],
                       wp[0:32, WOUT_OFF:WOUT_OFF + 32],
                       S_g[cj][s3 + 1][0:32, :], pos=(0, 32 * s3))
                nc.scalar.activation(
                    out=stash_eo[:, cj * n:(cj + 1) * n], in_=lg,
                    func=AF.Exp, bias=fp[0:96, BOUT_OFF:BOUT_OFF + 1])
                nc.scalar.activation(
                    out=stash_lg[:, cj * n:(cj + 1) * n], in_=lg,
                    func=AF.Identity, bias=fp[0:96, BOUT_OFF:BOUT_OFF + 1])

            # ---- B2 (ln table) ----
            for cj in range(GROUP):
                ci = g0 + cj
                so = ps_step.tile([96, n], F32, tag="stp", name="so")
                mm(so, wp[0:96, BDO96_OFF:BDO96_OFF + 96],
                   stash_eo[:, cj * n:(cj + 1) * n])
                ls = sb_b2.tile([96, n], F32, tag="ls", name="ls")
                nc.scalar.activation(out=ls, in_=so, func=AF.Ln)
                res = sb_b2.tile([96, n], F32, tag="res", name="res")
                nc.vector.tensor_tensor(out=res,
                                        in0=stash_lg[:, cj * n:(cj + 1) * n],
                                        in1=ls, op=ALU.subtract)
                ot = ps_step.tile([128, nsub * 96], F32, tag="stp", name="ot")
                for i in range(nsub):
                    nc.tensor.transpose(
                        ot[:, 96 * i:96 * i + 96],
                        res[:, 128 * i:128 * i + 128],
                        fp[0:96, IDT_OFF:IDT_OFF + 96])
                ob = sb_b2.tile([128, nsub, FT, OD], F32, tag="ob", name="ob")
                ot4 = ot.rearrange("p (i s o) -> p i s o", s=FT, o=32)
                nc.scalar.copy(out=ob, in_=ot4[:, :, :, 0:OD])
                ov = out_d[ci * n:(ci + 1) * n].rearrange(
                    "(i p) s o -> p i s o", p=128)
                nc.sync.dma_start(out=ov, in_=ob)


_PROGRAM_CACHE: dict[int, bass.Bass] = {}
_LAST_EXEC_NS = None
_LAST_RESULTS = None


def _get_program(Bshard: int) -> bass.Bass:
    if Bshard not in _PROGRAM_CACHE:
        _PROGRAM_CACHE[Bshard] = build_program(Bshard)
    return _PROGRAM_CACHE[Bshard]


def kernel(**inputs) -> np.ndarray:
    import ml_dtypes
    h_enc = np.asarray(inputs["h_enc"], np.float32)
    B = h_enc.shape[0]
    Bshard = B // N_CORES
    wp, fpk = _pack_weights(
        inputs["Wq"], inputs["bq"], inputs["Wk"], inputs["bk"],
        inputs["Wv"], inputs["bv"], inputs["W_ih"], inputs["b_ih"],
        inputs["W_hh"], inputs["b_hh"], inputs["W_out"], inputs["b_out"])
    wp_bf = wp.astype(ml_dtypes.bfloat16)
    nc = _get_program(Bshard)
    in_maps = []
    for c in range(N_CORES):
        in_maps.append({
            "h_enc": np.ascontiguousarray(h_enc[c * Bshard:(c + 1) * Bshard]),
            "wpack": wp_bf,
            "fpack": fpk,
        })
    import os
    trace = bool(os.environ.get("BASS_TRACE"))
    res = run_bass_kernel_spmd(nc, in_maps, list(range(N_CORES)), trace=trace)
    global _LAST_EXEC_NS, _LAST_RESULTS
    _LAST_EXEC_NS = res.exec_time_ns
    _LAST_RESULTS = res
    outs = [np.asarray(res.results[c]["out"]).reshape(Bshard, FT, OD)
            for c in range(N_CORES)]
    return np.concatenate(outs, axis=0).astype(np.float32)
